# revision 3
# baseline (speedup 1.0000x reference)
"""Trainium2 Bass kernel for the bilevel logit-assignment flow problem.

Reference math (N=384, cutoff-2 paths):
    A = (adj > 0) & ~eye
    E = A * exp(-lam * dist)                       # "edge weight" matrix
    Z = E + offdiag(E @ E)                          # softmax denominator (m cancels)
    W = where(Z > 0, demand / Z, 0),  demand = relu(od) (diag auto-zero via Z)
    flows = W*E + E*(W @ E^T) + E*(E^T @ W)

Sharding: origin axis o split across 8 cores (48 rows each). Each core
holds full E / E^T (N x N is small), computes its row-slice of Z/W and
the three matmuls, and returns:
    rows [48,384] = E_s * (W_s + W_s @ E^T)        # terms 1+2, rows S
    p3   [384,384] = E * (E_s^T @ W_s)             # term 3 partial (sum over its o-slice)
Host gather: flows = sum_i p3_i; flows[S_i] += rows_i.
"""

import numpy as np

import concourse.bass as bass
import concourse.mybir as mybir
import concourse.tile as tile
from concourse import bacc
from concourse.bass_utils import run_bass_kernel_spmd
from concourse.masks import make_identity

N = 384
NCORES = 8
S = N // NCORES  # 48 origins per core
P = 128
NT = N // P  # 3 partition tiles

F32 = mybir.dt.float32
F32R = mybir.dt.float32r
I32 = mybir.dt.int32
Act = mybir.ActivationFunctionType
Alu = mybir.AluOpType

USE_F32R = False  # fp32r: 1 cyc/row matmul (vs 4 for fp32) but rounded operands


def build_program(lam: float, use_f32r: bool = USE_F32R) -> bass.Bass:
    nc = bacc.Bacc("TRN2", target_bir_lowering=False, debug=False, num_devices=NCORES)

    mmdt = F32R if use_f32r else F32

    def asmm(ap):
        """View an SBUF AP in the dtype fed to the tensor engine."""
        return ap.bitcast(F32R) if use_f32r else ap

    adj = nc.dram_tensor("adj", [N, N], I32, kind="ExternalInput")
    dist = nc.dram_tensor("dist", [N, N], F32, kind="ExternalInput")
    adj_s = nc.dram_tensor("adj_s", [S, N], I32, kind="ExternalInput")
    dist_s = nc.dram_tensor("dist_s", [S, N], F32, kind="ExternalInput")
    od_s = nc.dram_tensor("od_s", [S, N], F32, kind="ExternalInput")
    noteye_s = nc.dram_tensor("noteye_s", [S, N], F32, kind="ExternalInput")
    p3 = nc.dram_tensor("p3", [N, N], F32, kind="ExternalOutput")
    rows = nc.dram_tensor("rows", [S, N], F32, kind="ExternalOutput")

    with tile.TileContext(nc) as tc:
        with (
            tc.tile_pool(name="persist", bufs=1) as sb,
            tc.tile_pool(name="work", bufs=2) as work,
            tc.tile_pool(name="pst", bufs=2, space="PSUM") as pst,
            tc.tile_pool(name="psacc", bufs=1, space="PSUM") as psacc,
        ):
            ident = sb.tile([P, P], F32)
            make_identity(nc, ident[:])

            # ---- full E [row-tiles] and its transpose ET ----
            E = sb.tile([P, NT, N], F32)   # E[p, t, :] == E_full[128*t + p, :]
            ET = sb.tile([P, NT, N], F32)  # ET[p, u, :] == E_full[:, 128*u + p].T
            for t in range(NT):
                adj_t = work.tile([P, N], I32, tag="adj_t")
                dist_t = work.tile([P, N], F32, tag="dist_t")
                nc.sync.dma_start(adj_t[:], adj[P * t : P * (t + 1), :])
                nc.sync.dma_start(dist_t[:], dist[P * t : P * (t + 1), :])
                adjf_t = work.tile([P, N], F32, tag="adjf_t")
                nc.vector.tensor_copy(adjf_t[:], adj_t[:])  # int32 -> f32
                # zero global diagonal of the adjacency mask:
                # iota = 128*t + p - y == 0 exactly on the diagonal
                nc.gpsimd.affine_select(
                    out=adjf_t[:],
                    in_=adjf_t[:],
                    compare_op=Alu.not_equal,
                    fill=0.0,
                    base=P * t,
                    channel_multiplier=1,
                    pattern=[[-1, N]],
                )
                expd_t = work.tile([P, N], F32, tag="expd_t")
                nc.scalar.activation(expd_t[:], dist_t[:], Act.Exp, scale=-lam)
                nc.vector.tensor_mul(asmm(E[:, t, :]), adjf_t[:], expd_t[:])

            for t in range(NT):
                for u in range(NT):
                    tp = pst.tile([P, P], F32, tag="tp")
                    nc.tensor.transpose(tp[:], E[:, t, P * u : P * (u + 1)], ident[:])
                    nc.vector.tensor_copy(asmm(ET[:, u, P * t : P * (t + 1)]), tp[:])

            # ---- per-core row slice E_s ----
            adjs_t = work.tile([S, N], I32, tag="adjs_t")
            dists_t = work.tile([S, N], F32, tag="dists_t")
            ods_t = work.tile([S, N], F32, tag="ods_t")
            ne_t = sb.tile([S, N], F32)
            nc.sync.dma_start(adjs_t[:], adj_s[:, :])
            nc.sync.dma_start(dists_t[:], dist_s[:, :])
            nc.sync.dma_start(ods_t[:], od_s[:, :])
            nc.sync.dma_start(ne_t[:], noteye_s[:, :])

            Es = sb.tile([S, N], F32)
            adjsf = work.tile([S, N], F32, tag="adjsf")
            nc.vector.tensor_copy(adjsf[:], adjs_t[:])
            nc.vector.tensor_mul(adjsf[:], adjsf[:], ne_t[:])  # zero diag (core-dep)
            expds = work.tile([S, N], F32, tag="expds")
            nc.scalar.activation(expds[:], dists_t[:], Act.Exp, scale=-lam)
            nc.vector.tensor_mul(asmm(Es[:]), adjsf[:], expds[:])

            # E_s^T [N, S] as NT chunks of [128, S]
            EsT = sb.tile([P, NT, S], F32)
            for c in range(NT):
                tp2 = pst.tile([P, S], F32, tag="tp2")
                nc.tensor.transpose(tp2[:], Es[:, P * c : P * (c + 1)], ident[:S, :S])
                nc.vector.tensor_copy(asmm(EsT[:, c, :]), tp2[:])

            # ---- (i) EEs = (E @ E)[S, :] ----
            EEs = psacc.tile([S, N], F32, tag="EEs")
            for kc in range(NT):
                nc.tensor.matmul(
                    EEs[:],
                    asmm(EsT[:, kc, :]),
                    asmm(E[:, kc, :]),
                    start=(kc == 0),
                    stop=(kc == NT - 1),
                )

            # ---- Z, W ----
            Zs = sb.tile([S, N], F32)
            nc.vector.tensor_add(Zs[:], Es[:], EEs[:])
            nc.vector.tensor_mul(Zs[:], Zs[:], ne_t[:])  # offdiag()
            mask = work.tile([S, N], F32, tag="mask")
            nc.vector.tensor_single_scalar(mask[:], Zs[:], 0.0, Alu.is_gt)
            nc.vector.tensor_scalar_max(Zs[:], Zs[:], 1e-30)
            zinv = work.tile([S, N], F32, tag="zinv")
            nc.vector.reciprocal(zinv[:], Zs[:])
            dem = work.tile([S, N], F32, tag="dem")
            nc.vector.tensor_relu(dem[:], ods_t[:])
            W = sb.tile([S, N], F32)
            nc.vector.tensor_mul(W[:], dem[:], zinv[:])
            nc.vector.tensor_mul(asmm(W[:]), W[:], mask[:])

            # W^T [N, S] chunks
            WsT = sb.tile([P, NT, S], F32)
            for c in range(NT):
                tp2 = pst.tile([P, S], F32, tag="tp2")
                nc.tensor.transpose(tp2[:], W[:, P * c : P * (c + 1)], ident[:S, :S])
                nc.vector.tensor_copy(asmm(WsT[:, c, :]), tp2[:])

            # ---- (ii) T2 = W_s @ E^T ----
            T2 = psacc.tile([S, N], F32, tag="T2")
            for dc in range(NT):
                nc.tensor.matmul(
                    T2[:],
                    asmm(WsT[:, dc, :]),
                    asmm(ET[:, dc, :]),
                    start=(dc == 0),
                    stop=(dc == NT - 1),
                )
            rows_sb = work.tile([S, N], F32, tag="rows_sb")
            nc.vector.tensor_add(rows_sb[:], W[:], T2[:])
            nc.vector.tensor_mul(rows_sb[:], rows_sb[:], Es[:])
            nc.sync.dma_start(rows[:, :], rows_sb[:])

            # ---- (iii) P3 = E_s^T @ W_s, p3 = E * P3 ----
            for mt in range(NT):
                P3 = pst.tile([P, N], F32, tag="P3")
                nc.tensor.matmul(
                    P3[:],
                    asmm(Es[:, P * mt : P * (mt + 1)]),
                    asmm(W[:]),
                    start=True,
                    stop=True,
                )
                out_t = work.tile([P, N], F32, tag="out_t")
                nc.vector.tensor_mul(out_t[:], E[:, mt, :], P3[:])
                nc.sync.dma_start(p3[P * mt : P * (mt + 1), :], out_t[:])

    nc.compile()  # bacc register allocation / DCE / lowering
    return nc


_PROGRAM_CACHE: dict = {}


def _get_program(lam: float, use_f32r: bool = USE_F32R) -> bass.Bass:
    key = (lam, use_f32r)
    if key not in _PROGRAM_CACHE:
        _PROGRAM_CACHE[key] = build_program(lam, use_f32r)
    return _PROGRAM_CACHE[key]


def make_in_maps(od, adj, dist):
    in_maps = []
    for i in range(NCORES):
        sl = slice(S * i, S * (i + 1))
        ne = np.ones((S, N), np.float32)
        ne[np.arange(S), np.arange(S * i, S * i + S)] = 0.0
        in_maps.append(
            {
                "adj": adj,
                "dist": dist,
                "adj_s": np.ascontiguousarray(adj[sl]),
                "dist_s": np.ascontiguousarray(dist[sl]),
                "od_s": np.ascontiguousarray(od[sl]),
                "noteye_s": ne,
            }
        )
    return in_maps


def gather(results) -> np.ndarray:
    out = np.zeros((N, N), np.float32)
    for i in range(NCORES):
        out += results[i]["p3"]
        out[S * i : S * i + S] += results[i]["rows"]
    return out


def kernel(od, adj, dist, lambda_param, capacity=None, **_unused) -> np.ndarray:
    od = np.ascontiguousarray(np.asarray(od, dtype=np.float32))
    adj = np.ascontiguousarray(np.asarray(adj, dtype=np.int32))
    dist = np.ascontiguousarray(np.asarray(dist, dtype=np.float32))
    lam = float(np.asarray(lambda_param))
    nc = _get_program(lam)
    res = run_bass_kernel_spmd(nc, make_in_maps(od, adj, dist), list(range(NCORES)))
    return gather(res.results)


# revision 5
# speedup vs baseline: 1.1687x; 1.1687x over previous
"""Trainium2 Bass kernel for the bilevel logit-assignment flow problem.

Reference math (N=384, cutoff-2 paths):
    A = (adj > 0) & ~eye
    E = A * exp(-lam * dist)                       # "edge weight" matrix
    Z = E + offdiag(E @ E)                          # softmax denominator (m cancels)
    W = where(Z > 0, demand / Z, 0),  demand = relu(od) (diag auto-zero via Z)
    flows = W*E + E*(W @ E^T) + E*(E^T @ W)

Sharding: origin axis o split across 8 cores (48 rows each). Each core
holds full E / E^T (N x N is small), computes its row-slice of Z/W and
the three matmuls, and returns:
    rows [48,384] = E_s * (W_s + W_s @ E^T)        # terms 1+2, rows S
    p3   [384,384] = E * (E_s^T @ W_s)             # term 3 partial (sum over its o-slice)
Host gather: flows = sum_i p3_i; flows[S_i] += rows_i.
"""

import numpy as np

import concourse.bass as bass
import concourse.mybir as mybir
import concourse.tile as tile
from concourse import bacc
from concourse.bass_utils import run_bass_kernel_spmd
from concourse.masks import make_identity

N = 384
NCORES = 8
S = N // NCORES  # 48 origins per core
P = 128
NT = N // P  # 3 partition tiles

F32 = mybir.dt.float32
F32R = mybir.dt.float32r
I32 = mybir.dt.int32
Act = mybir.ActivationFunctionType
Alu = mybir.AluOpType

USE_F32R = True  # fp32r: 1 cyc/row matmul (vs 4 for fp32), operands f32r-rounded


def build_program(lam: float, use_f32r: bool = USE_F32R) -> bass.Bass:
    nc = bacc.Bacc(
        "TRN2",
        target_bir_lowering=False,
        debug=False,
        num_devices=NCORES,
        enable_asserts=False,
    )

    def asmm(ap):
        """View an SBUF AP in the dtype fed to the tensor engine."""
        return ap.bitcast(F32R) if use_f32r else ap

    adj = nc.dram_tensor("adj", [N, N], I32, kind="ExternalInput")
    dist = nc.dram_tensor("dist", [N, N], F32, kind="ExternalInput")
    # per-core slice pack: [adj_s bits, dist_s, od_s, noteye_s] as f32 planes
    aux_s = nc.dram_tensor("aux_s", [4, S, N], F32, kind="ExternalInput")
    p3 = nc.dram_tensor("p3", [N, N], F32, kind="ExternalOutput")
    rows = nc.dram_tensor("rows", [S, N], F32, kind="ExternalOutput")

    adj_r = adj.rearrange("(t p) n -> p t n", p=P)  # [128, 3, 384]
    dist_r = dist.rearrange("(t p) n -> p t n", p=P)
    aux_r = aux_s.rearrange("k s n -> s k n")  # [48, 4, 384]
    p3_r = p3.rearrange("(t p) n -> p t n", p=P)

    with tile.TileContext(nc) as tc:
        with (
            tc.tile_pool(name="persist", bufs=1) as sb,
            tc.tile_pool(name="work", bufs=2) as work,
            tc.tile_pool(name="pst", bufs=2, space="PSUM") as pst,
            tc.tile_pool(name="psacc", bufs=1, space="PSUM") as psacc,
        ):
            ident = sb.tile([P, P], F32)
            make_identity(nc, ident[:])
            ident_mm = sb.tile([P, P], F32)
            nc.vector.tensor_copy(asmm(ident_mm[:]), ident[:])

            # ---- loads (issue split across the two HWDGE engines) ----
            adj_t = work.tile([P, NT, N], I32, tag="adj_t")
            dist_t = work.tile([P, NT, N], F32, tag="dist_t")
            aux = sb.tile([S, 4, N], F32)
            nc.sync.dma_start(adj_t[:], adj_r)
            nc.scalar.dma_start(dist_t[:], dist_r)
            nc.sync.dma_start(aux[:], aux_r)

            # ---- full E and its transpose ET ----
            E = sb.tile([P, NT, N], F32)   # E[p, t, :] == E_full[128*t + p, :]
            ET = sb.tile([P, NT, N], F32)  # ET[p, u, :] == E_full[:, 128*u + p].T
            adjf_t = work.tile([P, NT, N], F32, tag="adjf_t")
            nc.vector.tensor_copy(adjf_t[:], adj_t[:])  # int32 -> f32
            # zero the global diagonal in one pass: iota = p + 128*t - y
            nc.gpsimd.affine_select(
                out=adjf_t[:],
                in_=adjf_t[:],
                compare_op=Alu.not_equal,
                fill=0.0,
                base=0,
                channel_multiplier=1,
                pattern=[[P, NT], [-1, N]],
            )
            expd_t = work.tile([P, NT, N], F32, tag="expd_t")
            nc.scalar.activation(expd_t[:], dist_t[:], Act.Exp, scale=-lam)
            nc.vector.tensor_mul(asmm(E[:]), adjf_t[:], expd_t[:])

            for t in range(NT):
                for u in range(NT):
                    tp = pst.tile([P, P], F32, tag="tp")
                    nc.tensor.transpose(
                        asmm(tp[:]),
                        asmm(E[:, t, P * u : P * (u + 1)]),
                        asmm(ident_mm[:]),
                    )
                    nc.vector.tensor_copy(asmm(ET[:, u, P * t : P * (t + 1)]), tp[:])

            # ---- per-core row slice E_s ----
            ne_t = aux[:, 3, :]
            Es = sb.tile([S, N], F32)
            adjsf = work.tile([S, N], F32, tag="adjsf")
            nc.vector.tensor_copy(adjsf[:], aux[:, 0, :].bitcast(I32))
            nc.vector.tensor_mul(adjsf[:], adjsf[:], ne_t)  # zero diag (core-dep)
            expds = work.tile([S, N], F32, tag="expds")
            nc.scalar.activation(expds[:], aux[:, 1, :], Act.Exp, scale=-lam)
            nc.vector.tensor_mul(asmm(Es[:]), adjsf[:], expds[:])

            # E_s^T [N, S] as NT chunks of [128, S]
            EsT = sb.tile([P, NT, S], F32)
            for c in range(NT):
                tp2 = pst.tile([P, S], F32, tag="tp2")
                nc.tensor.transpose(
                    asmm(tp2[:]),
                    asmm(Es[:, P * c : P * (c + 1)]),
                    asmm(ident_mm[:S, :S]),
                )
                nc.vector.tensor_copy(asmm(EsT[:, c, :]), tp2[:])

            # ---- (i) EEs = (E @ E)[S, :] ----
            EEs = psacc.tile([S, N], F32, tag="EEs")
            for kc in range(NT):
                nc.tensor.matmul(
                    EEs[:],
                    asmm(EsT[:, kc, :]),
                    asmm(E[:, kc, :]),
                    start=(kc == 0),
                    stop=(kc == NT - 1),
                )

            # ---- Z, W ----
            dem = work.tile([S, N], F32, tag="dem")
            nc.vector.tensor_relu(dem[:], aux[:, 2, :])
            Zs = sb.tile([S, N], F32)
            nc.vector.tensor_add(Zs[:], Es[:], EEs[:])
            nc.vector.tensor_mul(Zs[:], Zs[:], ne_t)  # offdiag()
            mask = work.tile([S, N], F32, tag="mask")
            nc.vector.tensor_single_scalar(mask[:], Zs[:], 0.0, Alu.is_gt)
            nc.vector.tensor_scalar_max(Zs[:], Zs[:], 1e-30)
            zinv = work.tile([S, N], F32, tag="zinv")
            nc.vector.reciprocal(zinv[:], Zs[:])
            nc.vector.tensor_mul(dem[:], dem[:], mask[:])
            W = sb.tile([S, N], F32)
            nc.vector.tensor_mul(asmm(W[:]), dem[:], zinv[:])

            # W^T [N, S] chunks
            WsT = sb.tile([P, NT, S], F32)
            for c in range(NT):
                tp2 = pst.tile([P, S], F32, tag="tp2")
                nc.tensor.transpose(
                    asmm(tp2[:]),
                    asmm(W[:, P * c : P * (c + 1)]),
                    asmm(ident_mm[:S, :S]),
                )
                nc.vector.tensor_copy(asmm(WsT[:, c, :]), tp2[:])

            # ---- (ii) T2 = W_s @ E^T ----
            T2 = psacc.tile([S, N], F32, tag="T2")
            for dc in range(NT):
                nc.tensor.matmul(
                    T2[:],
                    asmm(WsT[:, dc, :]),
                    asmm(ET[:, dc, :]),
                    start=(dc == 0),
                    stop=(dc == NT - 1),
                )
            rows_sb = work.tile([S, N], F32, tag="rows_sb")
            nc.vector.tensor_add(rows_sb[:], W[:], T2[:])
            nc.vector.tensor_mul(rows_sb[:], rows_sb[:], Es[:])
            nc.scalar.dma_start(rows[:, :], rows_sb[:])

            # ---- (iii) P3 = E_s^T @ W_s, p3 = E * P3 ----
            O = sb.tile([P, NT, N], F32)
            for mt in range(NT):
                P3 = pst.tile([P, N], F32, tag="P3")
                nc.tensor.matmul(
                    P3[:],
                    asmm(Es[:, P * mt : P * (mt + 1)]),
                    asmm(W[:]),
                    start=True,
                    stop=True,
                )
                nc.vector.tensor_mul(O[:, mt, :], E[:, mt, :], P3[:])
            nc.sync.dma_start(p3_r, O[:])

    nc.compile()  # bacc register allocation / DCE / lowering
    return nc


_PROGRAM_CACHE: dict = {}


def _get_program(lam: float, use_f32r: bool = USE_F32R) -> bass.Bass:
    key = (lam, use_f32r)
    if key not in _PROGRAM_CACHE:
        _PROGRAM_CACHE[key] = build_program(lam, use_f32r)
    return _PROGRAM_CACHE[key]


def make_in_maps(od, adj, dist):
    in_maps = []
    for i in range(NCORES):
        sl = slice(S * i, S * (i + 1))
        ne = np.ones((S, N), np.float32)
        ne[np.arange(S), np.arange(S * i, S * i + S)] = 0.0
        aux = np.stack(
            [
                adj[sl].view(np.float32),
                dist[sl],
                od[sl],
                ne,
            ]
        )
        in_maps.append({"adj": adj, "dist": dist, "aux_s": np.ascontiguousarray(aux)})
    return in_maps


def gather(results) -> np.ndarray:
    out = np.zeros((N, N), np.float32)
    for i in range(NCORES):
        out += results[i]["p3"]
        out[S * i : S * i + S] += results[i]["rows"]
    return out


def kernel(od, adj, dist, lambda_param, capacity=None, **_unused) -> np.ndarray:
    od = np.ascontiguousarray(np.asarray(od, dtype=np.float32))
    adj = np.ascontiguousarray(np.asarray(adj, dtype=np.int32))
    dist = np.ascontiguousarray(np.asarray(dist, dtype=np.float32))
    lam = float(np.asarray(lambda_param))
    nc = _get_program(lam)
    res = run_bass_kernel_spmd(nc, make_in_maps(od, adj, dist), list(range(NCORES)))
    return gather(res.results)


# revision 7
# speedup vs baseline: 1.2267x; 1.0496x over previous
"""Trainium2 Bass kernel for the bilevel logit-assignment flow problem.

Reference math (N=384, cutoff-2 paths):
    A = (adj > 0) & ~eye
    E = A * exp(-lam * dist)                       # "edge weight" matrix
    Z = E + offdiag(E @ E)                          # softmax denominator (m cancels)
    W = where(Z > 0, demand / Z, 0),  demand = relu(od) (diag auto-zero via Z)
    flows = W*E + E*(W @ E^T) + E*(E^T @ W)

Sharding: origin axis o split across 8 cores (48 rows each). Each core
holds full E / E^T (N x N is small), computes its row-slice of Z/W and
the three matmuls, and returns:
    rows [48,384] = E_s * (W_s + W_s @ E^T)        # terms 1+2, rows S
    p3   [384,384] = E * (E_s^T @ W_s)             # term 3 partial (sum over its o-slice)
Host gather: flows = sum_i p3_i; flows[S_i] += rows_i.

Input marshaling (host side, layout only): adjacency is repacked to
uint8 (binary matrix, 4x fewer DMA bytes) and adj/dist/p3 use a
partition-tiled [128, 3, 384] DRAM layout so each partition's DMA row
is contiguous.
"""

import numpy as np

import concourse.bass as bass
import concourse.mybir as mybir
import concourse.tile as tile
from concourse import bacc
from concourse.bass_utils import run_bass_kernel_spmd
from concourse.masks import make_identity

N = 384
NCORES = 8
S = N // NCORES  # 48 origins per core
P = 128
NT = N // P  # 3 partition tiles

F32 = mybir.dt.float32
F32R = mybir.dt.float32r
I32 = mybir.dt.int32
U8 = mybir.dt.uint8
Act = mybir.ActivationFunctionType
Alu = mybir.AluOpType

USE_F32R = True  # fp32r: 1 cyc/row matmul (vs 4 for fp32), operands f32r-rounded


def build_program(lam: float, use_f32r: bool = USE_F32R) -> bass.Bass:
    nc = bacc.Bacc(
        "TRN2",
        target_bir_lowering=False,
        debug=False,
        num_devices=NCORES,
        enable_asserts=False,
    )

    def asmm(ap):
        """View an SBUF AP in the dtype fed to the tensor engine."""
        return ap.bitcast(F32R) if use_f32r else ap

    # partition-tiled layouts: [p, t, n] == full[128*t + p, n]
    adj = nc.dram_tensor("adj_u8t", [P, NT, N], U8, kind="ExternalInput")
    dist = nc.dram_tensor("dist_t", [P, NT, N], F32, kind="ExternalInput")
    # per-core slice pack: [adj_s bits, dist_s, od_s, noteye_s] as f32 planes
    aux_s = nc.dram_tensor("aux_s", [4, S, N], F32, kind="ExternalInput")
    p3 = nc.dram_tensor("p3_t", [P, NT, N], F32, kind="ExternalOutput")
    rows = nc.dram_tensor("rows", [S, N], F32, kind="ExternalOutput")

    aux_r = aux_s.rearrange("k s n -> s k n")  # [48, 4, 384]

    with tile.TileContext(nc) as tc:
        with (
            tc.tile_pool(name="persist", bufs=1) as sb,
            tc.tile_pool(name="work", bufs=2) as work,
            tc.tile_pool(name="pst", bufs=2, space="PSUM") as pst,
            tc.tile_pool(name="psacc", bufs=1, space="PSUM") as psacc,
        ):
            ident = sb.tile([P, P], F32)
            make_identity(nc, ident[:])
            ident_mm = sb.tile([P, P], F32)
            nc.vector.tensor_copy(asmm(ident_mm[:]), ident[:])

            # ---- loads (issue split across the two HWDGE engines) ----
            adj_t = work.tile([P, NT, N], U8, tag="adj_t")
            dist_sb = work.tile([P, NT, N], F32, tag="dist_sb")
            aux = sb.tile([S, 4, N], F32)
            nc.sync.dma_start(adj_t[:], adj[:])
            nc.scalar.dma_start(dist_sb[:, 0, :], dist[:, 0, :])
            nc.sync.dma_start(aux[:], aux_r)
            nc.scalar.dma_start(dist_sb[:, 1, :], dist[:, 1, :])
            nc.sync.dma_start(dist_sb[:, 2, :], dist[:, 2, :])

            # ---- full E (pipelined per row-tile) and its transpose ET ----
            E = sb.tile([P, NT, N], F32)   # E[p, t, :] == E_full[128*t + p, :]
            ET = sb.tile([P, NT, N], F32)  # ET[p, u, :] == E_full[:, 128*u + p].T
            adjf = work.tile([P, NT, N], F32, tag="adjf")
            expd = work.tile([P, NT, N], F32, tag="expd")
            for t in range(NT):
                nc.vector.tensor_copy(adjf[:, t, :], adj_t[:, t, :])  # u8 -> f32
                # zero the global diagonal: iota = 128*t + p - y
                nc.gpsimd.affine_select(
                    out=adjf[:, t, :],
                    in_=adjf[:, t, :],
                    compare_op=Alu.not_equal,
                    fill=0.0,
                    base=P * t,
                    channel_multiplier=1,
                    pattern=[[-1, N]],
                )
                nc.scalar.activation(expd[:, t, :], dist_sb[:, t, :], Act.Exp, scale=-lam)
                nc.vector.tensor_mul(asmm(E[:, t, :]), adjf[:, t, :], expd[:, t, :])
                for u in range(NT):
                    tp = pst.tile([P, P], F32, tag="tp")
                    nc.tensor.transpose(
                        asmm(tp[:]),
                        asmm(E[:, t, P * u : P * (u + 1)]),
                        asmm(ident_mm[:]),
                    )
                    nc.vector.tensor_copy(asmm(ET[:, u, P * t : P * (t + 1)]), tp[:])

            # ---- per-core row slice E_s ----
            ne_t = aux[:, 3, :]
            Es = sb.tile([S, N], F32)
            adjsf = work.tile([S, N], F32, tag="adjsf")
            nc.vector.tensor_copy(adjsf[:], aux[:, 0, :].bitcast(I32))
            nc.vector.tensor_mul(adjsf[:], adjsf[:], ne_t)  # zero diag (core-dep)
            expds = work.tile([S, N], F32, tag="expds")
            nc.scalar.activation(expds[:], aux[:, 1, :], Act.Exp, scale=-lam)
            nc.vector.tensor_mul(asmm(Es[:]), adjsf[:], expds[:])

            # E_s^T [N, S] as NT chunks of [128, S]
            EsT = sb.tile([P, NT, S], F32)
            for c in range(NT):
                tp2 = pst.tile([P, S], F32, tag="tp2")
                nc.tensor.transpose(
                    asmm(tp2[:]),
                    asmm(Es[:, P * c : P * (c + 1)]),
                    asmm(ident_mm[:S, :S]),
                )
                nc.vector.tensor_copy(asmm(EsT[:, c, :]), tp2[:])

            # ---- (i) EEs = (E @ E)[S, :] ----
            EEs = psacc.tile([S, N], F32, tag="EEs")
            for kc in range(NT):
                nc.tensor.matmul(
                    EEs[:],
                    asmm(EsT[:, kc, :]),
                    asmm(E[:, kc, :]),
                    start=(kc == 0),
                    stop=(kc == NT - 1),
                )

            # ---- Z, W ----
            dem = work.tile([S, N], F32, tag="dem")
            nc.vector.tensor_relu(dem[:], aux[:, 2, :])
            Zs = sb.tile([S, N], F32)
            nc.vector.tensor_add(Zs[:], Es[:], EEs[:])
            nc.vector.tensor_mul(Zs[:], Zs[:], ne_t)  # offdiag()
            mask = work.tile([S, N], F32, tag="mask")
            nc.vector.tensor_single_scalar(mask[:], Zs[:], 0.0, Alu.is_gt)
            nc.vector.tensor_scalar_max(Zs[:], Zs[:], 1e-30)
            zinv = work.tile([S, N], F32, tag="zinv")
            nc.vector.reciprocal(zinv[:], Zs[:])
            nc.vector.tensor_mul(dem[:], dem[:], mask[:])
            W = sb.tile([S, N], F32)
            nc.vector.tensor_mul(asmm(W[:]), dem[:], zinv[:])

            # W^T [N, S] chunks
            WsT = sb.tile([P, NT, S], F32)
            for c in range(NT):
                tp2 = pst.tile([P, S], F32, tag="tp2")
                nc.tensor.transpose(
                    asmm(tp2[:]),
                    asmm(W[:, P * c : P * (c + 1)]),
                    asmm(ident_mm[:S, :S]),
                )
                nc.vector.tensor_copy(asmm(WsT[:, c, :]), tp2[:])

            # ---- (iii) P3 = E_s^T @ W_s, p3 = E * P3 (early, per-tile out) ----
            for mt in range(NT):
                P3 = pst.tile([P, N], F32, tag="P3")
                nc.tensor.matmul(
                    P3[:],
                    asmm(Es[:, P * mt : P * (mt + 1)]),
                    asmm(W[:]),
                    start=True,
                    stop=True,
                )
                out_t = work.tile([P, N], F32, tag="out_t")
                nc.vector.tensor_mul(out_t[:], E[:, mt, :], P3[:])
                eng = nc.sync if mt % 2 == 0 else nc.scalar
                eng.dma_start(p3[:, mt, :], out_t[:])

            # ---- (ii) T2 = W_s @ E^T ----
            T2 = psacc.tile([S, N], F32, tag="T2")
            for dc in range(NT):
                nc.tensor.matmul(
                    T2[:],
                    asmm(WsT[:, dc, :]),
                    asmm(ET[:, dc, :]),
                    start=(dc == 0),
                    stop=(dc == NT - 1),
                )
            rows_sb = work.tile([S, N], F32, tag="rows_sb")
            nc.vector.tensor_add(rows_sb[:], W[:], T2[:])
            nc.vector.tensor_mul(rows_sb[:], rows_sb[:], Es[:])
            nc.scalar.dma_start(rows[:, :], rows_sb[:])

    nc.compile()  # bacc register allocation / DCE / lowering
    return nc


_PROGRAM_CACHE: dict = {}


def _get_program(lam: float, use_f32r: bool = USE_F32R) -> bass.Bass:
    key = (lam, use_f32r)
    if key not in _PROGRAM_CACHE:
        _PROGRAM_CACHE[key] = build_program(lam, use_f32r)
    return _PROGRAM_CACHE[key]


def _tile_rows(x: np.ndarray) -> np.ndarray:
    """[384, N] row-major -> [128, 3, N] partition-tiled layout."""
    return np.ascontiguousarray(x.reshape(NT, P, -1).transpose(1, 0, 2))


def _untile_rows(x: np.ndarray) -> np.ndarray:
    """[128, 3, N] partition-tiled -> [384, N]."""
    return x.transpose(1, 0, 2).reshape(N, -1)


def make_in_maps(od, adj, dist):
    adj_u8t = _tile_rows(adj.astype(np.uint8))
    dist_t = _tile_rows(dist)
    in_maps = []
    for i in range(NCORES):
        sl = slice(S * i, S * (i + 1))
        ne = np.ones((S, N), np.float32)
        ne[np.arange(S), np.arange(S * i, S * i + S)] = 0.0
        aux = np.stack(
            [
                adj[sl].view(np.float32),
                dist[sl],
                od[sl],
                ne,
            ]
        )
        in_maps.append(
            {
                "adj_u8t": adj_u8t,
                "dist_t": dist_t,
                "aux_s": np.ascontiguousarray(aux),
            }
        )
    return in_maps


def gather(results) -> np.ndarray:
    out = np.zeros((N, N), np.float32)
    for i in range(NCORES):
        out += _untile_rows(results[i]["p3_t"])
        out[S * i : S * i + S] += results[i]["rows"]
    return out


def kernel(od, adj, dist, lambda_param, capacity=None, **_unused) -> np.ndarray:
    od = np.ascontiguousarray(np.asarray(od, dtype=np.float32))
    adj = np.ascontiguousarray(np.asarray(adj, dtype=np.int32))
    dist = np.ascontiguousarray(np.asarray(dist, dtype=np.float32))
    lam = float(np.asarray(lambda_param))
    nc = _get_program(lam)
    res = run_bass_kernel_spmd(nc, make_in_maps(od, adj, dist), list(range(NCORES)))
    return gather(res.results)
